# revision 41
# baseline (speedup 1.0000x reference)
"""Trainium2 Bass kernel for triple-head Bahdanau attention (nn_Attention_48258252537865).

Reference computation (S=8192, H2=1024, A=2048, E=768):
  for each head t in {pos, cardinal, headline}:
      u_t = sentence @ W_sent_t + b_sent_t + (ctx_t @ W_ctx_t + b_ctx_t)   [1,S,A]
      e_t = tanh(u_t) @ v_t + bv_t                                          [1,S]
      w_t = softmax(mask(e_t))
  fused = (w_p + w_c + w_h) / 3
  out = fused @ sentence                                                    [1,H2]

Strategy: sequence-parallel over 8 NeuronCores; each core handles S/8 rows and
emits per-head (Z, N) partial softmax sums which the host combines exactly.

Numerics (unchanged from the 181us baseline):
  - U_FP8_PAIRS k-tile PAIRS of the u contraction run as fp8e4 DoubleRow
    matmuls; the remaining k-tiles run in bf16 (end-to-end rel err ~1.7e-2
    vs the 2e-2 gate; all-fp8 would be ~2.0e-2, over the gate).
  - no-max softmax: |e| <= sum|v| ~ 36 so exp(e) fits fp32 easily; the host
    just sums per-core Z and N.
  - the u accumulation carries a uniform x16 scale (W*16 in bf16, or W*8
    and x*2 for the fp8 pairs) undone by the tanh activation's scale=1/16.

Schedule (vs the 181us baseline):
  - head: the first 3 j-tiles run k-stage-major (kp0 for all 3, kp1 for all
    3, ...) so ~7.7us of PE work overlaps the 1.5MB sentT stream instead of
    2.6us; sentT chunk-1 rides the gpsimd ring in stage order.  Warm-up
    matmuls use memset tiles (ones x zeros) so they start right after engine
    init instead of waiting for the first DMA.
  - tail: the old copy -> fp32 selector matmul -> 3-lane exp -> PE transpose
    -> copy -> numerator chain is replaced by: bf16 copy of the 4-group
    score PSUM, then per s-tile ONE matmul eT[s,t] = e3w_chunk.T @ sel4
    (group-sum + transpose in one op), exp on 128 lanes, and the numerator
    with Z folded in as a ones-column matmul.  Outputs DMA straight from
    PSUM.
"""

import numpy as np
from contextlib import ExitStack

S = 8192
H2 = 1024
A = 2048
NCORES = 8
NEG = -1.0e30

# Per-head number of u-contraction k-tile pairs (of KT//2 = 4) computed in
# fp8e4 with DoubleRow (2x PE throughput); the remaining k-tiles run in bf16.
# One head runs all-fp8: measures max/max 1.72e-2, L2 1.82e-2 vs the 2e-2
# gate; (4,4,4) would be ~11us faster but its L2/mean error (2.03/2.07e-2)
# sits over the gate.  The all-fp8 head goes LAST: the first head overlaps
# the DMA-limited ramp where extra PE speed is wasted, and its k6/k7 fp8
# stream can land after the bf16 sentT groups.
PAIRS = (3, 4, 3)

_cache = {}
LAST_RESULTS = None  # BassKernelResults of the most recent device run


def _build(S_local, pairs_t):
    import concourse.bacc as bacc
    import concourse.tile as tile
    from concourse import mybir

    F32 = mybir.dt.float32
    BF16 = mybir.dt.bfloat16
    FP8 = mybir.dt.float8e4
    DR = mybir.MatmulPerfMode.DoubleRow
    TANH = mybir.ActivationFunctionType.Tanh
    EXP = mybir.ActivationFunctionType.Exp

    KT = H2 // 128                      # contraction k-tiles for u
    NK8_t = [2 * p for p in pairs_t]    # per-head k-tiles in fp8
    NKB_t = [KT - n for n in NK8_t]     # per-head k-tiles in bf16
    NK8M = max(NK8_t)
    NKBM = max(NKB_t)
    KB0 = KT - NKBM                     # first bf16-resident k-tile
    NJ = A // 128                       # a-tiles per head
    ST = S_local // 128                 # s-tiles (epilogue)
    SC = [(c, min(512, S_local - c)) for c in range(0, S_local, 512)]

    nc = bacc.Bacc("TRN2", target_bir_lowering=False, debug=False,
                   num_devices=NCORES)

    # sentT / sent arrive pre-interleaved to partition-major [128, (k s)]
    # contiguous blocks: ONE dma_start per block (each trigger costs ~650ns
    # of serial ring-engine time, and contiguous HBM reads coalesce).
    G8 = [(g, min(2, NK8M - g)) for g in range(0, NK8M, 2)]    # fp8 k-groups
    GB = [(g, min(2, NKBM - g)) for g in range(0, NKBM, 2)]    # bf16 k-groups
    sT8g_d = [nc.dram_tensor(f"sT8g{i}", [128, gn * S_local], FP8,
                             kind="ExternalInput") for i, (g, gn) in enumerate(G8)]
    sTbg_d = [nc.dram_tensor(f"sTbg{i}", [128, gn * S_local], BF16,
                             kind="ExternalInput") for i, (g, gn) in enumerate(GB)]
    if NK8M:
        W8_d = nc.dram_tensor("W8", [3, NJ, 128, NK8M * 128], FP8,
                              kind="ExternalInput")
        W8h_d = nc.dram_tensor("W8h", [128, 3 * NK8_t[0] * 128], FP8,
                               kind="ExternalInput")
    if NKBM:
        Wb_d = nc.dram_tensor("Wb", [3, NJ, 128, NKBM * 128], BF16,
                              kind="ExternalInput")
    if NKB_t[0]:
        Wbh_d = nc.dram_tensor("Wbh", [128, 3 * NKB_t[0] * 128], BF16,
                               kind="ExternalInput")
    sent_d = nc.dram_tensor("sent", [128, ST * H2], BF16, kind="ExternalInput")
    Vt_d = nc.dram_tensor("Vt", [128, 3 * NJ * 4], BF16, kind="ExternalInput")
    Bt_d = nc.dram_tensor("Bt", [128, 3 * NJ], F32, kind="ExternalInput")
    mask_d = nc.dram_tensor("mask1", [1, S_local], BF16, kind="ExternalInput")
    sel4_d = nc.dram_tensor("sel4", [128, 4], BF16, kind="ExternalInput")

    # N and Z share one output tensor (one DMA trigger): cols [0,H2) = N,
    # col H2 = Z
    Ncore_d = nc.dram_tensor("Ncore", [3, H2 + 4], F32, kind="ExternalOutput")

    with tile.TileContext(nc) as tc, ExitStack() as ctx:
        const = ctx.enter_context(tc.tile_pool(name="const", bufs=1))
        wpool = ctx.enter_context(tc.tile_pool(name="w", bufs=12))
        thpool = ctx.enter_context(tc.tile_pool(name="th", bufs=6))
        # phase-1 PSUM pools (all 8 banks); closed in stages before the
        # epilogue pools open so the banks can be reused
        ups = ExitStack()
        eps = ExitStack()
        epool = eps.enter_context(tc.tile_pool(name="e", bufs=1, space="PSUM"))
        upool = ups.enter_context(tc.tile_pool(name="u", bufs=3, space="PSUM"))

        # ---- memset consts first: the PE warm-up burst depends only on
        # these, so it starts right after engine init (no DMA wait) ----
        ones_sb = const.tile([1, 128], BF16, tag="ones")
        zrow_sb = const.tile([1, 512], BF16, tag="zrow")
        onescol_sb = const.tile([128, 4], BF16, tag="onescol")
        nc.any.memset(ones_sb[:], 1.0)
        nc.any.memset(zrow_sb[:], 0.0)
        nc.any.memset(onescol_sb[:], 1.0)

        # ---- score accumulator: 4 col-tiled groups, head t of group g on
        # partition 32g+t; batches of 4 score matmuls target distinct 32-col
        # PE groups so they stream concurrently.  One full-partition mask
        # matmul opens the accumulation and zero-adding closers end it. ----
        NG = 4
        assert NJ % NG == 0
        e3_ps = epool.tile([128, S_local], F32, tag="e")

        # ---- PE warm-up: the first ~5us are engine-init + DMA-bound and the
        # idle PE throttles to 1.2GHz; a burst of self-contained matmuls on
        # memset tiles keeps the HAM window busy so the array is warm when
        # the real stream begins.  start=True overwrites, and the real mask
        # matmul later start=True-overwrites the same region. ----
        nwarm, cw = 8, min(512, S_local)
        for _ in range(nwarm):
            nc.tensor.matmul(e3_ps[0:128, 0:cw], ones_sb[:], zrow_sb[0:1, 0:cw],
                             start=True, stop=True)

        # ---- head DMA: few big contiguous transfers.  sync ring: packed
        # head-tile weights + fp8 sentT groups; gpsimd ring: bf16 sentT
        # groups + the numerator operand.  The first HEADN j-tiles then run
        # k-stage-major so the PE streams while sentT lands. ----
        Wt_sb = {}

        def _wdma(t, j):
            tiles = []
            if NK8_t[t]:
                w8 = wpool.tile([128, NK8_t[t] * 128], FP8, tag="w8")
                nc.sync.dma_start(w8[:], W8_d.ap()[t, j, :, :NK8_t[t] * 128])
                tiles.append(w8)
            else:
                tiles.append(None)
            if NKB_t[t]:
                wb = wpool.tile([128, NKB_t[t] * 128], BF16, tag="wb")
                nc.sync.dma_start(wb[:], Wb_d.ap()[t, j, :, :NKB_t[t] * 128])
                tiles.append(wb)
            else:
                tiles.append(None)
            Wt_sb[(t, j)] = tiles

        HEADN = 3                       # head tiles == upool bufs
        head_tiles = [(0, j) for j in range(HEADN)]
        # sentT groups in head-consumption order: fp8 groups the FIRST head
        # uses, then its bf16 groups, then fp8 groups only later heads need
        if NK8M:
            sT8_sb = const.tile([128, NK8M * S_local], FP8, tag="sT8")
            w8h_all = const.tile([128, 3 * NK8_t[0] * 128], FP8, tag="w8h")
            nc.sync.dma_start(w8h_all[:], W8h_d.ap()[:])

        def _s8dma(i):
            g, gn = G8[i]
            nc.sync.dma_start(
                sT8_sb[:, g * S_local:(g + gn) * S_local], sT8g_d[i].ap()[:])

        for i, (g, gn) in enumerate(G8):
            if g < NK8_t[0]:
                _s8dma(i)
        if NKBM:
            sTb_sb = const.tile([128, NKBM * S_local], BF16, tag="sTb")
        if NKB_t[0]:
            wbh_all = const.tile([128, 3 * NKB_t[0] * 128], BF16, tag="wbh")
            nc.sync.dma_start(wbh_all[:], Wbh_d.ap()[:])
        if NKBM:
            for i, (g, gn) in enumerate(GB):
                nc.sync.dma_start(
                    sTb_sb[:, g * S_local:(g + gn) * S_local], sTbg_d[i].ap()[:])
        for i, (g, gn) in enumerate(G8):
            if g >= NK8_t[0]:
                _s8dma(i)
        # prefetch the next two steady tiles
        _wdma(0, HEADN)
        _wdma(0, HEADN + 1)

        # ---- consts on the scalar HWDGE ring (separate FIFO) ----
        Vt_sb = const.tile([128, 3 * NJ * 4], BF16, tag="vt")
        Bt_sb = const.tile([128, 3 * NJ], F32, tag="bt")
        mask_sb = const.tile([1, S_local], BF16, tag="mask")
        sel4_sb = const.tile([128, 4], BF16, tag="sel4")
        nc.scalar.dma_start(Bt_sb[:], Bt_d.ap()[:])
        nc.scalar.dma_start(Vt_sb[:], Vt_d.ap()[:])
        nc.scalar.dma_start(mask_sb[:], mask_d.ap()[:])
        nc.scalar.dma_start(sel4_sb[:], sel4_d.ap()[:])

        # ---- the big numerator operand isn't needed until the epilogue; its
        # transfers are emitted mid-steady-loop on the sync ring so its
        # descriptors don't compete with the head-critical sentT stream
        # (all queues share the same 16 DMA engines) ----
        sent_sb = const.tile([128, ST * H2], BF16, tag="sent")

        # ---- three heads: u -> tanh -> scores ----
        pend = []    # tanh tiles awaiting score matmuls (flushed 4 at a time)

        def _flush_scores():
            for (c, n) in SC:
                for g, (th_, tt, jj) in enumerate(pend):
                    nc.tensor.matmul(
                        e3_ps[32 * g:32 * g + 3, c:c + n],
                        Vt_sb[:, 4 * (jj * 3 + tt): 4 * (jj * 3 + tt) + 3],
                        th_[:, c:c + n],
                        start=False, stop=False,
                        tile_position=(0, 32 * g))
            pend.clear()

        def _u_fp8(u_ps, w8t, off, kp, c, n, start, stop):
            w8v = w8t[:, off + kp * 256: off + (kp + 1) * 256].rearrange(
                "p (i m) -> p i m", i=2)
            s8v = sT8_sb[:].rearrange("p (k s) -> p k s", k=NK8M)
            nc.tensor.matmul(u_ps[:, c:c + n], w8v,
                             s8v[:, 2 * kp:2 * kp + 2, c:c + n],
                             start=start, stop=stop, perf_mode=DR)

        def _u_bf16(u_ps, wbt, off, kb, kg, c, n, start, stop):
            # kb: k-tile index within this head's wb block; kg: global k-tile
            kt = kg - KB0   # index into the bf16-resident sentT
            nc.tensor.matmul(u_ps[:, c:c + n],
                             wbt[:, off + kb * 128: off + (kb + 1) * 128],
                             sTb_sb[:, kt * S_local + c: kt * S_local + c + n],
                             start=start, stop=stop)

        def _tanh(u_ps, t, j, chunked):
            th = thpool.tile([128, S_local], BF16, tag="th")
            if chunked:
                for (c, n) in SC:
                    nc.scalar.activation(
                        th[:, c:c + n], u_ps[:, c:c + n], TANH,
                        scale=1.0 / 16.0,
                        bias=Bt_sb[:, j * 3 + t: j * 3 + t + 1])
            else:
                nc.scalar.activation(th[:], u_ps[:], TANH, scale=1.0 / 16.0,
                                     bias=Bt_sb[:, j * 3 + t: j * 3 + t + 1])
            pend.append((th, t, j))

        # head tiles: k-stage-major (all HEADN tiles per k-stage)
        u_head = []
        for ti in range(HEADN):
            uh = upool.tile([128, S_local], F32, tag="u", name=f"uh{ti}")
            u_head.append(uh)
        p0, NK80, NKB0 = pairs_t[0], NK8_t[0], NKB_t[0]
        for kp in range(p0):
            for ti in range(HEADN):
                for (c, n) in SC:
                    _u_fp8(u_head[ti], w8h_all, ti * NK80 * 128, kp, c, n,
                           start=(kp == 0), stop=(kp == p0 - 1 and NKB0 == 0))
        for kb in range(NKB0):
            for ti in range(HEADN):
                for (c, n) in SC:
                    _u_bf16(u_head[ti], wbh_all, ti * NKB0 * 128, kb,
                            NK80 + kb, c, n,
                            start=(kb == 0 and p0 == 0),
                            stop=(kb == NKB0 - 1))
        for ti, (t, j) in enumerate(head_tiles):
            _tanh(u_head[ti], t, j, False)
            if t == 0 and j == 0:
                # additive key mask enters the score accumulator via a K=1
                # ones-matmul before every score matmul
                for (c, n) in SC:
                    nc.tensor.matmul(e3_ps[0:128, c:c + n], ones_sb[:],
                                     mask_sb[0:1, c:c + n],
                                     start=True, stop=False)

        # steady tiles
        NSENT = 2
        sent_cols = ST * H2
        sent_chunk = -(-sent_cols // NSENT)
        for t in range(3):
            for j in range(NJ):
                if t == 0 and j < HEADN:
                    continue
                if t == 1 and j % 8 == 0 and NSENT:
                    # slot a numerator-operand chunk into the W stream
                    si = j // 8
                    c0s = si * sent_chunk
                    c1s = min(sent_cols, c0s + sent_chunk)
                    if c0s < c1s:
                        nc.sync.dma_start(sent_sb[:, c0s:c1s],
                                          sent_d.ap()[:, c0s:c1s])
                w8, wb = Wt_sb.pop((t, j), (None, None))
                pt, nk8, nkb = pairs_t[t], NK8_t[t], NKB_t[t]
                if nk8 and w8 is None:
                    w8 = wpool.tile([128, nk8 * 128], FP8, tag="w8")
                    nc.sync.dma_start(w8[:], W8_d.ap()[t, j, :, :nk8 * 128])
                if nkb and wb is None:
                    wb = wpool.tile([128, nkb * 128], BF16, tag="wb")
                    nc.sync.dma_start(wb[:], Wb_d.ap()[t, j, :, :nkb * 128])
                u_ps = upool.tile([128, S_local], F32, tag="u")
                for kp in range(pt):
                    for (c, n) in SC:
                        _u_fp8(u_ps, w8, 0, kp, c, n,
                               start=(kp == 0),
                               stop=(kp == pt - 1 and nkb == 0))
                for kb in range(nkb):
                    for (c, n) in SC:
                        _u_bf16(u_ps, wb, 0, kb, nk8 + kb, c, n,
                                start=(kb == 0 and pt == 0),
                                stop=(kb == nkb - 1))
                if len(pend) == NG:
                    _flush_scores()
                _tanh(u_ps, t, j, chunked=(t == 2 and j == NJ - 1))
        # preload the Exp activation table while the PE finishes the last
        # score matmuls (the table swap costs ~1.7us on the ScalarE and would
        # otherwise land on the serial epilogue path)
        expwarm = const.tile([1, 3], F32, tag="expwarm")
        nc.scalar.activation(expwarm[:], ones_sb[0:1, 0:3], EXP)

        _flush_scores()
        # close the accumulation group across all 128 partitions (adds zeros)
        for (c, n) in SC:
            nc.tensor.matmul(e3_ps[0:128, c:c + n], ones_sb[:],
                             zrow_sb[0:1, 0:n], start=False, stop=True)

        # ---- fused epilogue: copy the 4-group accumulator to SBUF (bf16),
        # then per s-tile ONE matmul does group-sum + transpose at once:
        #   eT[s, t] = sum_p e3w[p, s] * sel4[p, t]   (sel4[32g+t, t] = 1)
        # exp then runs on all 128 partitions, and the numerator/Z follow.
        # (masked columns carry -1e30 on every partition -> eT = -4e30 ->
        # exp -> 0, exactly as the old selector path.) ----
        e3w_sb = const.tile([128, S_local], BF16, tag="e3w")
        e3x_sb = const.tile([128, 4 * ST], BF16, tag="e3x")
        ups.close()  # free the 6 u banks; epool (2) stays for the copies
        trs = ExitStack()
        trpool = trs.enter_context(tc.tile_pool(name="tr", bufs=3, space="PSUM"))

        CPY = 256   # copy granularity: lets eT matmuls start early
        eT_ps = []
        for c in range(0, S_local, CPY):
            n = min(CPY, S_local - c)
            nc.vector.tensor_copy(e3w_sb[:, c:c + n], e3_ps[:, c:c + n])
            for k in range(c // 128, (c + n) // 128):
                tp = trpool.tile([128, 4], F32, tag="tr")
                nc.tensor.matmul(tp[:, 0:3], e3w_sb[:, k * 128:(k + 1) * 128],
                                 sel4_sb[:, 0:3], start=True, stop=True)
                eT_ps.append(tp)
                if len(eT_ps) > 2:
                    # exp with lag 2 so trpool (bufs=3) cycles
                    kk = len(eT_ps) - 3
                    nc.scalar.activation(e3x_sb[:, 4 * kk:4 * kk + 3],
                                         eT_ps[kk][:, 0:3], EXP)
        for kk in range(max(0, ST - 2), ST):
            nc.scalar.activation(e3x_sb[:, 4 * kk:4 * kk + 3],
                                 eT_ps[kk][:, 0:3], EXP)

        trs.close()  # LIFO: tr, then the score-accumulator banks
        eps.close()
        npool = ctx.enter_context(tc.tile_pool(name="n", bufs=3, space="PSUM"))

        # ---- numerator + Z: N[t, :] = sum_s x[t, s] * sent[s, :],
        # Z[t] = sum_s x[t, s] via a ones-column matmul on the same
        # stationary ----
        n_ps = []
        for hi in range(H2 // 512):
            nt = npool.tile([3, 512], F32, tag="n", name=f"n{hi}")
            n_ps.append(nt)
        z_ps = npool.tile([3, 4], F32, tag="z")
        for k in range(ST):
            st = e3x_sb[:, 4 * k:4 * k + 3]
            for hi, hc in enumerate(range(0, H2, 512)):
                nc.tensor.matmul(n_ps[hi][0:3, :], st,
                                 sent_sb[:, k * H2 + hc: k * H2 + hc + 512],
                                 start=(k == 0), stop=(k == ST - 1))
            nc.tensor.matmul(z_ps[0:3, 0:4], st, onescol_sb[:],
                             start=(k == 0), stop=(k == ST - 1))
        n_sb = const.tile([3, H2 + 4], F32, tag="nsb")
        for hi, hc in enumerate(range(0, H2, 512)):
            nc.vector.tensor_copy(n_sb[:, hc:hc + 512], n_ps[hi][0:3, :])
        nc.vector.tensor_copy(n_sb[:, H2:H2 + 4], z_ps[0:3, 0:4])
        nc.sync.dma_start(Ncore_d.ap()[:], n_sb[:])

    nc.compile()
    return nc


def kernel(**inputs):
    global LAST_RESULTS
    import ml_dtypes
    from concourse import bass_utils

    E4 = ml_dtypes.float8_e4m3
    BF = ml_dtypes.bfloat16

    sentence = np.ascontiguousarray(
        np.asarray(inputs["sentence"], dtype=np.float32)[0])      # [S, H2]
    length = int(np.asarray(inputs["length"]).reshape(-1)[0])
    if length <= 0:
        return np.zeros((1, H2), dtype=np.float32)
    length = min(length, S)

    ctxs = [inputs["pos_embedding"], inputs["cardinal_phrase_embedding"],
            inputs["headline_embedding"]]
    tags = ["p", "c", "h"]

    # host-side prep: fold ctx projection + b_sent into a single bias [3, A]
    bias_all = np.empty((3, A), dtype=np.float32)
    W_all = np.empty((3, H2, A), dtype=np.float32)
    v_all = np.empty((3, A), dtype=np.float32)
    for i, tg in enumerate(tags):
        ctx = np.asarray(ctxs[i], dtype=np.float32)[0]            # [E]
        bias_all[i] = (np.asarray(inputs[f"b_sent_{tg}"], dtype=np.float32)
                       + ctx @ np.asarray(inputs[f"W_ctx_{tg}"], dtype=np.float32)
                       + np.asarray(inputs[f"b_ctx_{tg}"], dtype=np.float32))
        W_all[i] = np.asarray(inputs[f"W_sent_{tg}"], dtype=np.float32)
        v_all[i] = np.asarray(inputs[f"v_{tg}"], dtype=np.float32)

    pairs_t = PAIRS
    KT = H2 // 128
    NK8_t = [2 * p for p in pairs_t]
    NKB_t = [KT - n for n in NK8_t]
    NK8M = max(NK8_t)
    NKBM = max(NKB_t)
    KB0 = KT - NKBM
    S_local = max(128, -(-length // (NCORES * 128)) * 128)        # ceil, 128-aligned
    nc = _cache.get((S_local, pairs_t))
    if nc is None:
        nc = _build(S_local, pairs_t)
        _cache[(S_local, pairs_t)] = nc

    NJ = A // 128
    # W tiles, k-tile major per (t, j):  [3, NJ, 128, KT, 128] with the
    # partition dim holding the low 7 bits of the contraction index
    Wt = (W_all.reshape(3, KT, 128, NJ, 128)
               .transpose(0, 3, 2, 1, 4))                         # [3,NJ,128,KT,128]
    if NK8M:
        # fp8 pairs carry W*8 (and x*2) for a uniform x16 PSUM scale;
        # per-head blocks padded to NK8M (the pad is never transferred)
        W8 = np.zeros((3, NJ, 128, NK8M * 128), dtype=E4)
        for t in range(3):
            if NK8_t[t]:
                W8[t, :, :, :NK8_t[t] * 128] = np.clip(
                    Wt[t, :, :, :NK8_t[t]] * 8.0, -240, 240
                ).astype(E4).reshape(NJ, 128, NK8_t[t] * 128)
        # packed head-tile weights: tiles (0, 0..2) side by side, contiguous
        W8h = np.ascontiguousarray(
            W8[0, 0:3, :, :NK8_t[0] * 128].transpose(1, 0, 2)
            .reshape(128, 3 * NK8_t[0] * 128))
    if NKBM:
        Wb = np.zeros((3, NJ, 128, NKBM * 128), dtype=BF)
        for t in range(3):
            if NKB_t[t]:
                Wb[t, :, :, :NKB_t[t] * 128] = (
                    Wt[t, :, :, NK8_t[t]:] * 16.0
                ).astype(BF).reshape(NJ, 128, NKB_t[t] * 128)
    if NKB_t[0]:
        Wbh = np.ascontiguousarray(
            Wb[0, 0:3, :, :NKB_t[0] * 128].transpose(1, 0, 2)
            .reshape(128, 3 * NKB_t[0] * 128))

    def _interleave(rows, nk):
        # [nk*128, S_local] -> partition-major [128, nk*S_local], contiguous
        return np.ascontiguousarray(
            rows.reshape(nk, 128, -1).transpose(1, 0, 2).reshape(128, -1))

    # [128, (j t) * 3]: head t's v-tile in column t of its [128, 3] block
    vt_cols = v_all.T.reshape(NJ, 128, 3).transpose(1, 0, 2)      # [128, NJ, 3]
    Vt = np.zeros((128, NJ, 3, 4), dtype=np.float32)
    for t in range(3):
        Vt[:, :, t, t] = vt_cols[:, :, t]
    Vt = np.ascontiguousarray(Vt.reshape(128, 3 * NJ * 4)).astype(BF)
    Bt = np.ascontiguousarray(
        bias_all.T.reshape(NJ, 128, 3).transpose(1, 0, 2).reshape(128, 3 * NJ))
    sel4 = np.zeros((128, 4), dtype=np.float32)
    for g in range(4):
        for t in range(3):
            sel4[32 * g + t, t] = 1.0
    sel4 = sel4.astype(BF)

    in_maps = []
    for c in range(NCORES):
        s0 = c * S_local
        sl = sentence[s0:s0 + S_local]
        if sl.shape[0] < S_local:                                  # pad tail core
            sl = np.concatenate(
                [sl, np.zeros((S_local - sl.shape[0], H2), np.float32)], axis=0)
        mask1 = np.where((s0 + np.arange(S_local))[None, :] < length,
                         0.0, NEG).astype(np.float32).astype(BF)
        slT = sl.T                                                 # [H2, S_local]
        ST = S_local // 128
        sent_i = np.ascontiguousarray(
            sl.astype(BF).reshape(ST, 128, H2).transpose(1, 0, 2)
            .reshape(128, ST * H2))
        im = dict(Vt=Vt, Bt=Bt, mask1=mask1, sel4=sel4, sent=sent_i)
        if NK8M:
            sT8 = np.clip(slT[:NK8M * 128] * 2.0, -240, 240).astype(E4)
            for i in range(0, NK8M, 2):
                gn = min(2, NK8M - i)
                im[f"sT8g{i // 2}"] = _interleave(
                    sT8[i * 128:(i + gn) * 128], gn)
            im["W8"] = W8
            im["W8h"] = W8h
        if NKBM:
            sTb = slT[KB0 * 128:].astype(BF)
            for i in range(0, NKBM, 2):
                gn = min(2, NKBM - i)
                im[f"sTbg{i // 2}"] = _interleave(
                    sTb[i * 128:(i + gn) * 128], gn)
            im["Wb"] = Wb
        if NKB_t[0]:
            im["Wbh"] = Wbh
        in_maps.append(im)

    res = bass_utils.run_bass_kernel_spmd(nc, in_maps,
                                          core_ids=list(range(NCORES)))
    LAST_RESULTS = res

    # ---- exact cross-core softmax combine: plain sums (no max shift) ----
    Z = np.zeros(3, dtype=np.float64)
    N = np.zeros((3, H2), dtype=np.float64)
    for c in range(NCORES):
        nc_out = res.results[c]["Ncore"].astype(np.float64)
        Z += nc_out[:, H2]
        N += nc_out[:, :H2]
    out = (N / Z[:, None]).mean(axis=0)
    return out[None, :].astype(np.float32)
